# revision 24
# baseline (speedup 1.0000x reference)
"""CapsNet dynamic-routing kernel for 8 TRN2 NeuronCores.

Problem: x [256,1152,8], W [1152,10,8,16], 3 routing iterations, out [256,10,16,1].

Strategy v3 (warm-up collective + replicated first iteration + I-sharded rest):
  The first collective of an execution pays a ~36-43us ncfw wake measured
  from ITS DOORBELL (not from execution start).  So a 4-byte warm-up
  AllReduce fires at ~11us (as soon as gpsimd is up); the wake runs
  concurrently with a long collective-free front:
    Iteration 1 uses uniform coupling c=1/O, so s1 = x_flat @ W_flat needs
    no routing state: every core computes the FULL s1 redundantly (144
    matmuls, ~23us, gated by the 7.7MB xT/W load) instead of AllReduce-ing
    partials.  Then v1 = squash(s1), agreement G = xf_local^T @ v1 (K=256),
    local b/c, and the local s2 partial (K=1152).
  The s2 AllReduce (82KB bf16, split per batch-tile into two pipelined
  41KB collectives) then runs at warm speed, as does the final iteration's
  ReduceScatter.  Host-side tile permutation (roll by 9*rank) puts each
  core's local (i,d)-tiles at positions [0:9] of the full 72-tile xT/Wb
  tensors, so the SPMD program needs no rank arithmetic.
  Matmul operands bf16; PSUM accumulation fp32; collectives carry bf16.
  Softmax skips max-subtraction: |b| stays O(1) for this routing.
"""

import numpy as np

B, I, O, DIN, DOUT = 256, 1152, 10, 8, 16
NCORES = 8
I_SH = I // NCORES          # 144 input capsules per core
ID = I_SH * DIN             # 1152 local (i,d) rows
NT = ID // 128              # 9 local partition tiles of (i,d)
NTF = (I * DIN) // 128      # 72 full partition tiles of (i,d)
BT = B // 128               # 2 partition tiles of batch
OE = O * DOUT               # 160
ROUTING_ITERS = 3
PSH = 128 // NCORES         # 16 output partitions per core (ReduceScatter)
LCH = 6                     # setup DMA chunk size (tiles) for xT/Wb loads

_CACHE = {}


def _bc(ap_mod, ap, n):
    """View an AP with an extra innermost broadcast axis of length n."""
    return ap_mod.AP(tensor=ap.tensor, offset=ap.offset, ap=[*ap.ap, [0, n]])


def _build():
    import concourse.bass as bass
    import concourse.bacc as bacc
    import concourse.tile as tile
    from concourse import mybir

    f32 = mybir.dt.float32
    bf16 = mybir.dt.bfloat16
    AF = mybir.ActivationFunctionType
    ALU = mybir.AluOpType

    nc = bacc.Bacc("TRN2", target_bir_lowering=False, debug=False,
                   num_devices=NCORES)

    xT_d = nc.dram_tensor("xT", [128, NTF, B], bf16, kind="ExternalInput")
    xf_d = nc.dram_tensor("xf", [128, BT, ID], bf16, kind="ExternalInput")
    Wb_d = nc.dram_tensor("Wb", [128, NTF, OE], bf16, kind="ExternalInput")
    MB_d = nc.dram_tensor("Mblk", [128, 128], bf16, kind="ExternalInput")
    out_d = nc.dram_tensor("out", [PSH, BT, OE], f32, kind="ExternalOutput")

    with tile.TileContext(nc) as tc:
        with (
            tc.tile_pool(name="sb", bufs=1) as sb,
            tc.tile_pool(name="work", bufs=2) as work,
            tc.tile_pool(name="ps_s", bufs=2, space="PSUM") as ps_s,
            tc.tile_pool(name="ps_g", bufs=3, space="PSUM") as ps_g,
            tc.tile_pool(name="ps_a", bufs=2, space="PSUM") as ps_a,
            tc.tile_pool(name="dram", bufs=4, space="DRAM") as dram,
        ):
            # Warm-up collective: 512B AllReduce fired as soon as gpsimd is
            # up.  Its only job is to absorb the ~40us first-collective ncfw
            # wake while the collective-free front below runs.
            wu_in = nc.inline_tensor(
                np.zeros((1, 128), dtype=np.float32), name="wu_in")
            wu_out = dram.tile([NCORES, 128], f32, tag="wu_out",
                               addr_space="Shared")
            nc.gpsimd.collective_compute(
                "AllGather", ALU.bypass,
                replica_groups=[list(range(NCORES))],
                ins=[wu_in.ap().opt()], outs=[wu_out.opt()])

            # ---- persistent SBUF tensors ----
            xT = sb.tile([128, NTF, B], bf16)     # full x_flat^T (lhsT for s)
            xf = sb.tile([128, BT, ID], bf16)     # local x_flat (lhsT for G)
            Wb = sb.tile([128, NTF, OE], bf16)    # full W_flat bf16
            Mblk = sb.tile([128, 128], bf16)      # 8x8 block-diag ones
            bq = sb.tile([128, NT, O], f32)       # local routing logits b
            Wc = sb.tile([128, NT, OE], bf16)     # local c * W
            s_sb = sb.tile([128, BT, OE], bf16)   # local partial s (iters 2+)
            sf = sb.tile([128, BT, OE], bf16)     # all-reduced s (iters 2+)
            sf1 = sb.tile([128, BT, OE], f32)     # full s1 (iter 1, no AR)
            vb = sb.tile([128, BT, OE], bf16)     # squash(s) bf16 (rhs for G)
            WG = sb.tile([128, NT, OE], bf16)     # W * G (local)
            A1 = sb.tile([128, NT, O], bf16)      # e-reduced agreement
            dmy = sb.tile([128, 1], f32)          # ACT table-prefetch scratch

            # Setup loads: spread xT/Wb chunks over all three DMA-capable
            # engines (sync, scalar, gpsimd — warmup collective already
            # rang) with two tiny lead-in chunks so the first s1 matmuls
            # start as early as possible; a chunk's xT and Wb halves go to
            # different queues so each k-tile pair lands together.
            qs = [nc.sync, nc.scalar, nc.gpsimd]
            nc.gpsimd.dma_start(out=Mblk[:], in_=MB_d[:])
            bounds = [0, 2, 4] + list(range(4 + LCH, NTF, LCH)) + [NTF]
            for ci in range(len(bounds) - 1):
                c0, c1 = bounds[ci], bounds[ci + 1]
                qs[ci % 3].dma_start(out=xT[:, c0:c1], in_=xT_d[:, c0:c1])
                qs[(ci + 1) % 3].dma_start(out=Wb[:, c0:c1],
                                           in_=Wb_d[:, c0:c1])
            nc.gpsimd.dma_start(out=xf[:], in_=xf_d[:])

            # Pre-load the Sqrt ACT table during setup (needs a non-negative
            # input: Mblk is all 0/1 and lands first on gpsimd's queue).
            nc.scalar.activation(out=dmy[:], in_=Mblk[:, 0:1], func=AF.Sqrt)

            Wb4 = Wb.rearrange("p t (o e) -> p t o e", o=O)
            Wc4 = Wc.rearrange("p t (o e) -> p t o e", o=O)
            sf4 = sf.rearrange("p b (o e) -> p b o e", o=O)
            sf14 = sf1.rearrange("p b (o e) -> p b o e", o=O)
            vb4 = vb.rearrange("p b (o e) -> p b o e", o=O)

            for it in range(ROUTING_ITERS):
                first, last = it == 0, it == ROUTING_ITERS - 1

                # list of (bt, sin_view, gate) producing squash input per bt
                if first:
                    # s1 = x_flat @ W_flat over the FULL K=9216 (c=1/O
                    # folded into the squash scale).  Replicated; no
                    # collective.  k-major so each freshly-DMA'd chunk is
                    # consumed once for both batch tiles.
                    s1_ps = [ps_s.tile([128, OE], f32, name=f"s1_{bt}",
                                       tag="s_ps") for bt in range(BT)]
                    for k in range(NTF):
                        for bt in range(BT):
                            nc.tensor.matmul(
                                s1_ps[bt][:],
                                xT[:, k, bt * 128:(bt + 1) * 128],
                                Wb[:, k, :],
                                start=(k == 0), stop=(k == NTF - 1))
                    for bt in range(BT):
                        nc.vector.tensor_copy(sf1[:, bt, :], s1_ps[bt][:])
                    sin = [sf14[:, bt] for bt in range(BT)]
                else:
                    # c = softmax(b) over o per local (i,d) row.
                    ex = work.tile([128, NT, O], f32, tag="ex")
                    nc.scalar.activation(out=ex[:], in_=bq[:], func=AF.Exp)
                    nc.scalar.activation(out=dmy[:], in_=ex[:, 0, 0:1],
                                         func=AF.Sqrt)
                    sm = work.tile([128, NT], f32, tag="sm")
                    nc.vector.reduce_sum(out=sm[:], in_=ex[:],
                                         axis=mybir.AxisListType.X)
                    nc.vector.reciprocal(out=sm[:], in_=sm[:])
                    nc.vector.tensor_tensor(
                        out=ex[:], in0=ex[:], in1=_bc(bass, sm[:], O),
                        op=ALU.mult)
                    GRP = 3
                    for g in range(0, NT, GRP):
                        # middle chunk on gpsimd: overlaps Vector's first
                        # chunk so the s-matmuls stream sooner
                        eng = nc.gpsimd if g == GRP else nc.vector
                        eng.tensor_tensor(
                            out=Wc4[:, g:g + GRP],
                            in0=_bc(bass, ex[:, g:g + GRP, :], DOUT),
                            in1=Wb4[:, g:g + GRP], op=ALU.mult)

                    # s_partial = x_local @ Wc : out [b-tile 128, OE].  One
                    # collective for both batch tiles — each collective
                    # costs ~11-12us of serialized CC processing regardless
                    # of payload, so fewer is better.
                    cc_in = dram.tile([128, BT, OE], bf16,
                                      name=f"ci{it}", tag="cc_in")
                    for bt in range(BT):
                        s_ps = ps_s.tile([128, OE], f32, name=f"s2_{bt}",
                                         tag="s_ps")
                        for k in range(NT):
                            nc.tensor.matmul(
                                s_ps[:],
                                xT[:, k, bt * 128:(bt + 1) * 128],
                                Wc[:, k, :],
                                start=(k == 0), stop=(k == NT - 1))
                        nc.vector.tensor_copy(s_sb[:, bt, :], s_ps[:])
                        nc.sync.dma_start(out=cc_in[:, bt, :],
                                          in_=s_sb[:, bt, :])

                    if last:
                        cc_rs = dram.tile([PSH, BT, OE], bf16, tag="cc_rs")
                        nc.gpsimd.collective_compute(
                            "ReduceScatter", ALU.add,
                            replica_groups=[list(range(NCORES))],
                            ins=[cc_in.opt()], outs=[cc_rs.opt()])
                        s3 = sb.tile([PSH, BT, OE], bf16)
                        nc.sync.dma_start(out=s3[:], in_=cc_rs[:])
                        s34 = s3.rearrange("p b (o e) -> p b o e", o=O)
                        sq3 = work.tile([PSH, BT, OE], f32, tag="sq3")
                        nc.vector.tensor_tensor(out=sq3[:], in0=s3[:],
                                                in1=s3[:], op=ALU.mult)
                        ss3 = work.tile([PSH, BT, O], f32, tag="ss3")
                        nc.vector.reduce_sum(
                            out=ss3[:],
                            in_=sq3.rearrange("p b (o e) -> p b o e", o=O),
                            axis=mybir.AxisListType.X)
                        t13 = work.tile([PSH, BT, O], f32, tag="t13")
                        nc.scalar.activation(out=t13[:], in_=ss3[:],
                                             func=AF.Sqrt)
                        den3 = work.tile([PSH, BT, O], f32, tag="den3")
                        nc.vector.tensor_scalar_add(den3[:], ss3[:], 1.0)
                        nc.vector.reciprocal(out=den3[:], in_=den3[:])
                        rat3 = work.tile([PSH, BT, O], f32, tag="rat3")
                        nc.vector.tensor_tensor(out=rat3[:], in0=t13[:],
                                                in1=den3[:], op=ALU.mult)
                        v3 = work.tile([PSH, BT, OE], f32, tag="v3")
                        nc.vector.tensor_tensor(
                            out=v3.rearrange("p b (o e) -> p b o e", o=O),
                            in0=s34, in1=_bc(bass, rat3[:], DOUT),
                            op=ALU.mult)
                        nc.sync.dma_start(out=out_d[:], in_=v3[:])
                        continue

                    cc_out = dram.tile([128, BT, OE], bf16, tag="cc_out",
                                       addr_space="Shared")
                    nc.gpsimd.collective_compute(
                        "AllReduce", ALU.add,
                        replica_groups=[list(range(NCORES))],
                        ins=[cc_in.opt()], outs=[cc_out.opt()])
                    nc.sync.dma_start(out=sf[:], in_=cc_out[:])
                    sin = [sf4[:, bt] for bt in range(BT)]

                # squash per batch tile: v = s * sqrt(ss)/(1+ss) per (b, o);
                # iteration 1 carries c=1/O as s_raw = O*s_true.  The big
                # elementwise ops of bt1 run on gpsimd (Pool) in parallel
                # with bt0's on Vector; the tiny [128,O] recip chain stays
                # on Vector (gpsimd has no reciprocal).
                for bt in range(BT):
                    eng = nc.vector if bt == 0 else nc.gpsimd
                    sq = work.tile([128, OE], f32, tag="sq")
                    sq4 = sq.rearrange("p (o e) -> p o e", o=O)
                    eng.tensor_tensor(out=sq4[:], in0=sin[bt],
                                      in1=sin[bt], op=ALU.mult)
                    ss = work.tile([128, O], f32, tag="ss")
                    nc.vector.reduce_sum(out=ss[:], in_=sq4,
                                         axis=mybir.AxisListType.X)
                    t1 = work.tile([128, O], f32, tag="t1")
                    nc.scalar.activation(out=t1[:], in_=ss[:], func=AF.Sqrt)
                    den = work.tile([128, O], f32, tag="den")
                    if first:
                        nc.vector.tensor_scalar(
                            out=den[:], in0=ss[:], scalar1=1.0 / (O * O),
                            scalar2=1.0, op0=ALU.mult, op1=ALU.add)
                    else:
                        nc.vector.tensor_scalar_add(den[:], ss[:], 1.0)
                    nc.vector.reciprocal(out=den[:], in_=den[:])
                    rat = work.tile([128, O], f32, tag="rat")
                    if first:
                        # fold the c=1/O carry (1/O^2) into the rat multiply
                        nc.vector.scalar_tensor_tensor(
                            out=rat[:], in0=t1[:], scalar=1.0 / (O * O),
                            in1=den[:], op0=ALU.mult, op1=ALU.mult)
                    else:
                        nc.vector.tensor_tensor(out=rat[:], in0=t1[:],
                                                in1=den[:], op=ALU.mult)
                    eng.tensor_tensor(
                        out=vb4[:, bt], in0=sin[bt],
                        in1=_bc(bass, rat[:], DOUT), op=ALU.mult)
                # prefetch Exp table for the next softmax
                nc.scalar.activation(out=dmy[:], in_=vb[:, 0, 0:1],
                                     func=AF.Exp)

                # G = xf_local^T @ v ; agree = (1/B) sum_de W*G ; b += agree.
                # Three (i,d)-tiles share one PSUM bank; each (g,j) slot's
                # start/stop accumulation pair completes across both batch
                # tiles before the next opens (one pending group per bank).
                # shrinking group sizes so the last group's W*G multiply
                # and e-reduction (which gate the b update) are short
                # (must stay on Vector: gpsimd cannot read PSUM)
                for gi, (g, gw) in enumerate(((0, 3), (3, 3), (6, 2),
                                              (8, 1))):
                    eng = nc.vector
                    g_ps = ps_g.tile([128, gw, OE], f32, name=f"gp{g}",
                                     tag="g_ps")
                    for j in range(gw):
                        for bt in range(BT):
                            nc.tensor.matmul(
                                g_ps[:, j, :],
                                xf[:, bt, (g + j) * 128:(g + j + 1) * 128],
                                vb[:, bt, :],
                                start=(bt == 0), stop=(bt == BT - 1))
                    eng.tensor_tensor(
                        out=WG[:, g:g + gw, :], in0=Wb[:, g:g + gw, :],
                        in1=g_ps[:], op=ALU.mult)
                    with nc.allow_low_precision("agreement tolerates bf16"):
                        nc.vector.reduce_sum(
                            out=A1[:, g:g + gw, :],
                            in_=WG[:, g:g + gw, :].rearrange(
                                "p g (o e) -> p (g o) e", o=O),
                            axis=mybir.AxisListType.X)
                # single fused d-reduction matmul: [128,128] @ [128, NT*O]
                a_ps = ps_a.tile([128, NT, O], f32, tag="a_ps")
                nc.tensor.matmul(
                    a_ps.rearrange("p t o -> p (t o)"), Mblk[:],
                    A1.rearrange("p t o -> p (t o)"),
                    start=True, stop=True)
                if first:
                    nc.vector.tensor_scalar_mul(bq[:], a_ps[:], 1.0 / B)
                else:
                    nc.vector.scalar_tensor_tensor(
                        out=bq[:], in0=a_ps[:], scalar=1.0 / B,
                        in1=bq[:], op0=ALU.mult, op1=ALU.add)

    nc.compile()
    return nc


def _get_nc():
    if "nc" not in _CACHE:
        _CACHE["nc"] = _build()
    return _CACHE["nc"]


def _tile128(a):
    """[R, C] -> [128, R//128, C] with row r = t*128+p at [p, t]."""
    r, c = a.shape
    return np.ascontiguousarray(
        a.reshape(r // 128, 128, c).transpose(1, 0, 2))


def _make_in_maps(x, W):
    from concourse import mybir
    bfdt = mybir.dt.np(mybir.dt.bfloat16)
    x = np.asarray(x, dtype=np.float32)
    W = np.asarray(W, dtype=np.float32)
    mblk = np.kron(np.eye(16, dtype=np.float32),
                   np.ones((8, 8), dtype=np.float32)).astype(bfdt)
    x_flat = x.reshape(B, I * DIN)                         # [256, 9216]
    w_flat = W.transpose(0, 2, 1, 3).reshape(I * DIN, OE)  # [9216, 160]
    xT_t = np.ascontiguousarray(x_flat.T).reshape(NTF, 128, B)
    wb_t = w_flat.reshape(NTF, 128, OE)
    in_maps = []
    for core in range(NCORES):
        roll = np.roll(np.arange(NTF), -NT * core)
        xT_c = xT_t[roll].reshape(I * DIN, B)
        wb_c = wb_t[roll].reshape(I * DIN, OE)
        isl = slice(core * I_SH, (core + 1) * I_SH)
        xf_c = x[:, isl, :].reshape(B, ID)
        in_maps.append({
            "xT": _tile128(xT_c).astype(bfdt),
            "xf": _tile128(xf_c).astype(bfdt),
            "Wb": _tile128(wb_c).astype(bfdt),
            "Mblk": mblk,
        })
    return in_maps


def _ensure_ntff_hook():
    """This image's antenv lacks axon_hooks; reconstruct it so trace=True
    can reach the NTFF profiler in libaxon_pjrt.so."""
    import sys
    import types
    try:
        import antenv.axon_hooks  # noqa: F401
        return
    except ImportError:
        pass
    try:
        import antenv
        from trn_agent_boot.trn_boot import _ntff_profile_via_ctypes
        hook = _ntff_profile_via_ctypes("/opt/axon/libaxon_pjrt.so")
        mod = types.ModuleType("antenv.axon_hooks")
        mod._hook = hook
        mod.get_axon_ntff_profile_hook = lambda: mod._hook
        mod.set_axon_ntff_profile_hook = (
            lambda h: setattr(mod, "_hook", h))
        sys.modules["antenv.axon_hooks"] = mod
        antenv.axon_hooks = mod
    except Exception as e:  # profiling is best-effort
        print("ntff hook setup failed:", e)


def _run_hw(x, W, trace=False, **kwargs):
    from concourse import bass_utils
    if trace:
        _ensure_ntff_hook()
    nc = _get_nc()
    res = bass_utils.run_bass_kernel_spmd(
        nc, _make_in_maps(x, W), core_ids=list(range(NCORES)),
        trace=trace, **kwargs)
    shards = np.stack([res.results[c]["out"] for c in range(NCORES)])
    return _assemble(shards), res


def _assemble(shards):
    """shards [NCORES, 16, BT, OE] -> full [B, O, DOUT, 1]; core r's shard
    holds batch rows bt*128 + 16*r + p (ReduceScatter partition sharding)."""
    shards = np.asarray(shards, dtype=np.float32).reshape(
        NCORES, 128 // NCORES, BT, OE)
    # [r, p, bt, f] -> [bt, r, p, f] -> [B, OE]
    full = shards.transpose(2, 0, 1, 3).reshape(B, OE)
    return full.reshape(B, O, DOUT)[..., None]


def kernel(x, W):
    out, _ = _run_hw(x, W, trace=False)
    return out
